# revision 12
# baseline (speedup 1.0000x reference)
"""BiDAF attention layer on 8 Trainium2 NeuronCores (Bass/Tile), v2.

Math (per batch b):
  t[i,j]  = sum_d (c[i,d]*w_cq[d] + w_q[d]) * q[j,d]   (= cq + sq0[j])
  a       = softmax_j(t)            (biases b_c/b_q/b_cq cancel in softmax)
  c2q     = a @ q
  m[i]    = max_j t[i,j];  sc0[i] = c[i,:]@w_c
  bvec    = softmax_i(m + sc0)      (biases cancel here too)
  q2c     = bvec @ c
  out     = [c | c2q | c*c2q | c*q2c]

Sharding: data-parallel over batch, 4 batches per core, params replicated.

v2 changes vs the 136us baseline (all fp16-matmul, PE-bound):
  - Scores computed ONCE, only in the transposed [j,i] layout the c2q
    matmul needs. The row max over j (a partition-dim reduction there) is
    done as max_j t = log max_j e^t: chunk-max of e^T on DVE, 8 small PE
    transposes of the [j,512] partials, then a free-dim reduce_max. This
    kills the entire second score pass (24 N=512 matmuls per batch).
  - sc0 is computed directly in column layout by 16 tiny N=1 matmuls
    (lhsT = cT tile, rhs = w_c column), then folded into the bvec
    numerators as ebv = (max_j e^t) * exp(sc0 - 2.5).
  - c is loaded once, cast f32->fp16 in flight (SWDGE); no separate f32
    copy.  All PE work is fp16 (1 cycle/row).
  - The output is written as fp16 (tolerance is 2e-2; fp16 rounding is
    ~2e-4) and upcast to f32 on the host: halves the store traffic.
  - Output staged as one [P, NT, 3D] tile (c2q | c*c2q | c*q2c) so each
    half is a single store; block0 (= c) stores straight from c_sb.
  - DMA paths: loads on SWDGE (casts), block0 on the scalar HWDGE ring,
    stage stores on the sync ring (avoids head-of-line blocking between
    prefetched block0 stores and current-batch stage stores).
  - PE emission order interleaves transposes / score matmuls / c2q
    matmuls / q2c chain so the PE never idles waiting on ACT exp or DVE
    evacuations (idle gaps also drop the PE out of its 2.4GHz p-state).
"""

import sys

if "/opt/trn_rl_repo" not in sys.path:
    sys.path.insert(0, "/opt/trn_rl_repo")

import numpy as np

import concourse.bass as bass
import concourse.tile as tile
from concourse import bacc, bass_isa, mybir
from concourse.bass import ds, ts
from concourse.masks import make_identity

B, CL, QL, D = 32, 1024, 512, 256
NCORES = 8
BS = B // NCORES  # batches per core
P = 128
F32 = mybir.dt.float32
F16 = mybir.dt.float16

NT = CL // P  # 8 i-tiles
NJ = QL // P  # 4 j-chunks
ND = D // P   # 2 d-chunks
NH = 2        # i-halves for the [j,i]-layout score matmul
IH = CL // NH  # 512
KPH = NT // NH  # i-tiles per half

Exp = mybir.ActivationFunctionType.Exp
AxX = mybir.AxisListType.X
Mult = mybir.AluOpType.mult
Add = mybir.AluOpType.add


def build_bass(bs: int = BS):
    nc = bacc.Bacc(None)
    c_d = nc.declare_dram_parameter("c", [bs, CL, D], F32, isOutput=False)
    q_d = nc.declare_dram_parameter("q", [bs, QL, D], F32, isOutput=False)
    wc_d = nc.declare_dram_parameter("wc_cols", [P, ND], F16, isOutput=False)
    wq_d = nc.declare_dram_parameter("wq_cols", [P, ND], F32, isOutput=False)
    wcq_d = nc.declare_dram_parameter("wcq_cols", [P, ND], F32, isOutput=False)
    out_d = nc.declare_dram_parameter("out", [bs, CL, 4 * D], F16, isOutput=True)

    D2, D3 = 2 * D, 3 * D

    with tile.TileContext(nc) as tc:
        with (
            tc.tile_pool(name="consts", bufs=1) as consts,
            tc.tile_pool(name="ins", bufs=3) as ins,
            tc.tile_pool(name="work", bufs=2) as work,
            tc.tile_pool(name="stg", bufs=2) as stg,
            tc.tile_pool(name="ps_mm", bufs=2, space="PSUM") as ps_mm,
            tc.tile_pool(name="ps_tr", bufs=2, space="PSUM") as ps_tr,
            tc.tile_pool(name="ps_s", bufs=3, space="PSUM") as ps_s,
            tc.tile_pool(name="ps_q", bufs=1, space="PSUM") as ps_q,
        ):
            ident_h = consts.tile([P, P], F16)
            ones_f = consts.tile([P, P], F32)
            ones_h1 = consts.tile([1, P], F16)
            wc_sb = consts.tile([P, ND], F16)
            wq_sb = consts.tile([P, ND], F32)
            wcq_sb = consts.tile([P, ND], F32)
            neg_shift = consts.tile([P, 1], F32)

            # params on the sync ring, identity/memsets on DVE+gpsimd --
            # all overlap the first SWDGE loads
            nc.sync.dma_start(out=wc_sb, in_=wc_d[:])
            nc.sync.dma_start(out=wq_sb, in_=wq_d[:])
            nc.sync.dma_start(out=wcq_sb, in_=wcq_d[:])

            def emit_inputs(b, chunked):
                q_sb = ins.tile([P, NJ, D + 1], F16, tag="q_sb")
                qv = q_d[b].rearrange("(t p) d -> p t d", p=P)
                if chunked:
                    nc.gpsimd.dma_start(out=q_sb[:, 0:2, 0:D], in_=qv[:, 0:2])
                    nc.gpsimd.dma_start(out=q_sb[:, 2:4, 0:D], in_=qv[:, 2:4])
                else:
                    nc.gpsimd.dma_start(out=q_sb[:, :, 0:D], in_=qv)
                nc.vector.memset(q_sb[:, :, D : D + 1], 1.0)
                c_sb = ins.tile([P, NT, D], F16, tag="c_sb")
                cv = c_d[b].rearrange("(t p) d -> p t d", p=P)
                nc.gpsimd.dma_start(out=c_sb, in_=cv)
                ov = out_d[b].rearrange("(t p) x -> p t x", p=P)
                return c_sb, q_sb, ov

            make_identity(nc, ident_h)
            nc.vector.memset(ones_f, 1.0)
            nc.vector.memset(ones_h1, 1.0)
            nc.vector.memset(neg_shift, -2.5)
            pending = [emit_inputs(0, True)]

            for b in range(bs):
                c_sb, q_sb, ov = pending.pop(0)
                # prefetch up to two batches ahead
                if b == 0:
                    for nb in (1, 2):
                        if nb < bs:
                            pending.append(emit_inputs(nb, False))
                elif b + 2 < bs:
                    pending.append(emit_inputs(b + 2, False))

                # block0 (= c) stores straight from c_sb; issued here (not at
                # prefetch time) so the sync queue never waits on a future
                # batch's load in front of this batch's stores
                nc.sync.dma_start(out=ov[:, :, 0:D], in_=c_sb)

                # ---------------- transposes (PE, fp16) ----------------
                # qT: both d-chunks into one [P,1024] PSUM, single DVE evac
                qT = work.tile([P, ND, QL], F16, tag="qT")
                psq = ps_tr.tile([P, ND, QL], F16, tag="tr")
                for dc in range(ND):
                    for jc in range(NJ):
                        nc.tensor.transpose(
                            psq[:, dc, ts(jc, P)], q_sb[:, jc, ts(dc, P)], ident_h
                        )
                nc.vector.tensor_copy(qT, psq)

                # c: per d-chunk both halves into one [P,1024] PSUM;
                # chatT (affine) evac on DVE, plain cT evac on ACT
                cT = work.tile([P, ND, CL], F16, tag="cT")
                chatT = work.tile([P, ND, CL], F16, tag="chatT")
                for dc in range(ND):
                    pst = ps_tr.tile([P, CL], F16, tag="tr")
                    for it in range(NT):
                        nc.tensor.transpose(
                            pst[:, ts(it, P)], c_sb[:, it, ts(dc, P)], ident_h
                        )
                    nc.vector.tensor_scalar(
                        out=chatT[:, dc],
                        in0=pst,
                        scalar1=wcq_sb[:, dc : dc + 1],
                        scalar2=wq_sb[:, dc : dc + 1],
                        op0=Mult,
                        op1=Add,
                    )
                    nc.scalar.copy(cT[:, dc], pst)

                # ---------------- scores: tT + exp ----------------
                eT0 = work.tile([P, NJ, IH], F16, tag="eT0")
                eT1 = work.tile([P, NJ, IH], F16, tag="eT1")
                eT = [eT0, eT1]

                def score_chunk(h, jc):
                    pmm = ps_mm.tile([P, IH], F32, tag="tT")
                    for dc in range(ND):
                        nc.tensor.matmul(
                            pmm,
                            qT[:, dc, ts(jc, P)],
                            chatT[:, dc, ds(h * IH, IH)],
                            start=(dc == 0),
                            stop=(dc == ND - 1),
                        )
                    nc.scalar.activation(eT[h][:, jc], pmm, Exp)

                for jc in range(NJ):
                    score_chunk(0, jc)

                for jc in range(NJ):
                    score_chunk(1, jc)
                    if jc == 1:
                        # row max over j, stage 1: max across j-chunks (DVE)
                        M1a0 = work.tile([P, 2, IH], F16, tag="m1a0")
                        nc.vector.tensor_max(
                            M1a0, eT[0][:, 0:2, :], eT[0][:, 2:4, :]
                        )
                        M1h0 = work.tile([P, IH], F16, tag="m1h0")
                        nc.vector.tensor_max(M1h0, M1a0[:, 0, :], M1a0[:, 1, :])

                # sc0 in column layout: 16 tiny N=1 matmuls.  Emitted after
                # the h1 scores so the cT evacuation (ACT) has plenty of
                # slack before the first LDWEIGHTS reads it.
                psc = ps_tr.tile([P, NT], F32, tag="tr")
                for it in range(NT):
                    for dc in range(ND):
                        nc.tensor.matmul(
                            psc[:, it : it + 1],
                            cT[:, dc, ts(it, P)],
                            wc_sb[:, dc : dc + 1],
                            start=(dc == 0),
                            stop=(dc == ND - 1),
                        )
                e_sc0 = work.tile([P, NT], F16, tag="esc0")
                nc.scalar.activation(e_sc0, psc, Exp, bias=neg_shift[:, 0:1])

                # ------------ c2q + row-max + q2c, interleaved ------------
                stage = stg.tile([P, NT, D2], F16, tag="stage")
                c4st = stg.tile([P, NT, D], F16, tag="c4st")
                linv = work.tile([P, NT], F32, tag="linv")
                Me16 = work.tile([P, NT], F16, tag="me")

                def mm2_tile(h, k):
                    it = h * KPH + k
                    po = ps_s.tile([P, D + 1], F32, tag="po")
                    for jc in range(NJ):
                        nc.tensor.matmul(
                            po,
                            eT[h][:, jc, ts(k, P)],
                            q_sb[:, jc],
                            start=(jc == 0),
                            stop=(jc == NJ - 1),
                        )
                    nc.vector.reciprocal(linv[:, it : it + 1], po[:, D : D + 1])
                    nc.scalar.mul(
                        stage[:, it, 0:D], po[:, 0:D], linv[:, it : it + 1]
                    )

                def m1t(h, m1h_tile):
                    # stage 2: transpose the [j,512] partial maxes, then a
                    # free-dim reduce gives m in column layout
                    ptm = ps_tr.tile([P, KPH, P], F16, tag="tr")
                    for k in range(KPH):
                        nc.tensor.transpose(
                            ptm[:, k, :], m1h_tile[:, ts(k, P)], ident_h
                        )
                    nc.vector.reduce_max(
                        Me16[:, h * KPH : (h + 1) * KPH], ptm, AxX
                    )

                mm2_tile(0, 0)
                m1t(0, M1h0)
                mm2_tile(0, 1)
                # chunk-max h1 (eT1 complete by now), then its transposes
                M1a1 = work.tile([P, 2, IH], F16, tag="m1a1")
                nc.vector.tensor_max(M1a1, eT[1][:, 0:2, :], eT[1][:, 2:4, :])
                M1h1 = work.tile([P, IH], F16, tag="m1h1")
                nc.vector.tensor_max(M1h1, M1a1[:, 0, :], M1a1[:, 1, :])
                m1t(1, M1h1)
                # bvec numerators: ebv = (max_j e^t) * e^(sc0-2.5)
                ebv = work.tile([P, NT], F16, tag="ebv")
                nc.vector.tensor_mul(ebv, Me16, e_sc0)
                colsum = work.tile([P, 1], F32, tag="colsum")
                nc.vector.reduce_sum(colsum, ebv, AxX)
                tot_sb = work.tile([P, 1], F32, tag="tot")
                nc.gpsimd.partition_all_reduce(
                    tot_sb, colsum, channels=P, reduce_op=bass_isa.ReduceOp.add
                )
                totinv = work.tile([P, 1], F32, tag="totinv")
                nc.vector.reciprocal(totinv, tot_sb)
                mm2_tile(0, 2)
                mm2_tile(0, 3)
                ps_q2c = ps_q.tile([1, D], F32, tag="q")
                for it in range(NT):
                    nc.tensor.matmul(
                        ps_q2c,
                        ebv[:, it : it + 1],
                        c_sb[:, it],
                        start=(it == 0),
                        stop=(it == NT - 1),
                    )
                q2c_row = work.tile([1, D], F16, tag="q2cr")
                nc.vector.tensor_scalar_mul(q2c_row, ps_q2c, totinv[0:1, 0:1])
                # c*c2q h0 (one strided DVE op over 4 tiles), store h0
                nc.vector.tensor_mul(
                    stage[:, 0:KPH, D:D2],
                    c_sb[:, 0:KPH],
                    stage[:, 0:KPH, 0:D],
                )
                nc.sync.dma_start(out=ov[:, 0:KPH, D:D3], in_=stage[:, 0:KPH])
                mm2_tile(1, 0)
                ps_q2cb = ps_q.tile([P, D], F32, tag="q")
                nc.tensor.matmul(
                    ps_q2cb, ones_h1, q2c_row, start=True, stop=True
                )
                q2c_sb = work.tile([P, D], F16, tag="q2csb")
                nc.scalar.copy(q2c_sb, ps_q2cb)
                mm2_tile(1, 1)

                # c*q2c: gpsimd, except split with DVE on the last batch
                # (parallel finish matters only at the tail)
                def c4_mul(it):
                    eng = (
                        nc.vector
                        if (b == bs - 1 and it % 2 == 1)
                        else nc.gpsimd
                    )
                    eng.tensor_mul(c4st[:, it], c_sb[:, it], q2c_sb)

                for it in range(KPH):
                    c4_mul(it)
                nc.sync.dma_start(
                    out=ov[:, 0:KPH, D3 : 4 * D], in_=c4st[:, 0:KPH]
                )
                mm2_tile(1, 2)
                for it in range(KPH, NT):
                    c4_mul(it)
                nc.sync.dma_start(
                    out=ov[:, KPH:NT, D3 : 4 * D], in_=c4st[:, KPH:NT]
                )
                mm2_tile(1, 3)
                nc.vector.tensor_mul(
                    stage[:, KPH:NT, D:D2],
                    c_sb[:, KPH:NT],
                    stage[:, KPH:NT, 0:D],
                )
                # last store on the scalar HWDGE ring so its transfer runs in
                # parallel with the c4 store on the sync ring at the tail
                nc.scalar.dma_start(out=ov[:, KPH:NT, D:D3], in_=stage[:, KPH:NT])

    nc.compile()
    return nc


_NC_CACHE = {}


def _get_nc(bs: int = BS):
    if bs not in _NC_CACHE:
        _NC_CACHE[bs] = build_bass(bs)
    return _NC_CACHE[bs]


def _param_maps(w_c, w_q, w_cq):
    wc_cols = np.ascontiguousarray(
        np.asarray(w_c, np.float32).reshape(ND, P).T.astype(np.float16)
    )
    wq_cols = np.ascontiguousarray(np.asarray(w_q, np.float32).reshape(ND, P).T)
    wcq_cols = np.ascontiguousarray(
        np.asarray(w_cq, np.float32).reshape(ND, P).T
    )
    return wc_cols, wq_cols, wcq_cols


def _run(c, q, w_c, w_q, w_cq, trace=False, **trace_kwargs):
    from concourse.bass_utils import run_bass_kernel_spmd

    c = np.asarray(c, np.float32)
    q = np.asarray(q, np.float32)
    wc_cols, wq_cols, wcq_cols = _param_maps(w_c, w_q, w_cq)

    nc = _get_nc(BS)
    in_maps = []
    for k in range(NCORES):
        in_maps.append(
            {
                "c": np.ascontiguousarray(c[k * BS : (k + 1) * BS]),
                "q": np.ascontiguousarray(q[k * BS : (k + 1) * BS]),
                "wc_cols": wc_cols,
                "wq_cols": wq_cols,
                "wcq_cols": wcq_cols,
            }
        )
    res = None
    last_err = None
    for attempt in range(3):
        try:
            res = run_bass_kernel_spmd(
                nc,
                in_maps,
                core_ids=list(range(NCORES)),
                trace=trace,
                **trace_kwargs,
            )
            break
        except Exception as e:  # transient device wedges clear on retry
            last_err = e
            if "UNRECOVERABLE" not in str(e) and "UNAVAILABLE" not in str(e):
                raise
    if res is None:
        raise last_err
    out = np.concatenate([res.results[k]["out"] for k in range(NCORES)], axis=0)
    return np.ascontiguousarray(out.astype(np.float32)), res


def kernel(c, q, w_c, b_c, w_q, b_q, w_cq, b_cq):
    # b_c/b_q/b_cq provably cancel in both softmaxes; output doesn't use them.
    out, _ = _run(c, q, w_c, w_q, w_cq)
    return out


# revision 13
# speedup vs baseline: 1.4375x; 1.4375x over previous
"""BiDAF attention layer on 8 Trainium2 NeuronCores (Bass/Tile), v2.

Math (per batch b):
  t[i,j]  = sum_d (c[i,d]*w_cq[d] + w_q[d]) * q[j,d]   (= cq + sq0[j])
  a       = softmax_j(t)            (biases b_c/b_q/b_cq cancel in softmax)
  c2q     = a @ q
  m[i]    = max_j t[i,j];  sc0[i] = c[i,:]@w_c
  bvec    = softmax_i(m + sc0)      (biases cancel here too)
  q2c     = bvec @ c
  out     = [c | c2q | c*c2q | c*q2c]

Sharding: data-parallel over batch, 4 batches per core, params replicated.

v2 changes vs the 136us baseline (all fp16-matmul, PE-bound):
  - Scores computed ONCE, only in the transposed [j,i] layout the c2q
    matmul needs. The row max over j (a partition-dim reduction there) is
    done as max_j t = log max_j e^t: chunk-max of e^T on DVE, 8 small PE
    transposes of the [j,512] partials, then a free-dim reduce_max. This
    kills the entire second score pass (24 N=512 matmuls per batch).
  - sc0 is computed directly in column layout by 16 tiny N=1 matmuls
    (lhsT = cT tile, rhs = w_c column), then folded into the bvec
    numerators as ebv = (max_j e^t) * exp(sc0 - 2.5).
  - c is loaded once, cast f32->fp16 in flight (SWDGE); no separate f32
    copy.  All PE work is fp16 (1 cycle/row).
  - The output is written as fp16 (tolerance is 2e-2; fp16 rounding is
    ~2e-4) and upcast to f32 on the host: halves the store traffic.
  - Output staged as one [P, NT, 3D] tile (c2q | c*c2q | c*q2c) so each
    half is a single store; block0 (= c) stores straight from c_sb.
  - DMA paths: loads on SWDGE (casts), block0 on the scalar HWDGE ring,
    stage stores on the sync ring (avoids head-of-line blocking between
    prefetched block0 stores and current-batch stage stores).
  - PE emission order interleaves transposes / score matmuls / c2q
    matmuls / q2c chain so the PE never idles waiting on ACT exp or DVE
    evacuations (idle gaps also drop the PE out of its 2.4GHz p-state).
"""

import sys

if "/opt/trn_rl_repo" not in sys.path:
    sys.path.insert(0, "/opt/trn_rl_repo")

import numpy as np

import concourse.bass as bass
import concourse.tile as tile
from concourse import bacc, bass_isa, mybir
from concourse.bass import ds, ts
from concourse.masks import make_identity

B, CL, QL, D = 32, 1024, 512, 256
NCORES = 8
BS = B // NCORES  # batches per core
P = 128
F32 = mybir.dt.float32
F16 = mybir.dt.float16

NT = CL // P  # 8 i-tiles
NJ = QL // P  # 4 j-chunks
ND = D // P   # 2 d-chunks
NH = 2        # i-halves for the [j,i]-layout score matmul
IH = CL // NH  # 512
KPH = NT // NH  # i-tiles per half

Exp = mybir.ActivationFunctionType.Exp
AxX = mybir.AxisListType.X
Mult = mybir.AluOpType.mult
Add = mybir.AluOpType.add


def build_bass(bs: int = BS):
    nc = bacc.Bacc(None)
    c_d = nc.declare_dram_parameter("c", [bs, CL, D], F32, isOutput=False)
    q_d = nc.declare_dram_parameter("q", [bs, QL, D], F32, isOutput=False)
    wc_d = nc.declare_dram_parameter("wc_cols", [P, ND], F16, isOutput=False)
    wq_d = nc.declare_dram_parameter("wq_cols", [P, ND], F32, isOutput=False)
    wcq_d = nc.declare_dram_parameter("wcq_cols", [P, ND], F32, isOutput=False)
    out_d = nc.declare_dram_parameter("out", [bs, CL, 4 * D], F16, isOutput=True)

    D2, D3 = 2 * D, 3 * D

    with tile.TileContext(nc) as tc:
        with (
            tc.tile_pool(name="consts", bufs=1) as consts,
            tc.tile_pool(name="ins", bufs=3) as ins,
            tc.tile_pool(name="work", bufs=2) as work,
            tc.tile_pool(name="stg", bufs=2) as stg,
            tc.tile_pool(name="ps_mm", bufs=2, space="PSUM") as ps_mm,
            tc.tile_pool(name="ps_tr", bufs=2, space="PSUM") as ps_tr,
            tc.tile_pool(name="ps_s", bufs=3, space="PSUM") as ps_s,
            tc.tile_pool(name="ps_q", bufs=1, space="PSUM") as ps_q,
        ):
            ident_h = consts.tile([P, P], F16)
            ones_f = consts.tile([P, P], F32)
            ones_h1 = consts.tile([1, P], F16)
            wc_sb = consts.tile([P, ND], F16)
            wq_sb = consts.tile([P, ND], F32)
            wcq_sb = consts.tile([P, ND], F32)
            neg_shift = consts.tile([P, 1], F32)

            # params on the sync ring, identity/memsets on DVE+gpsimd --
            # all overlap the first SWDGE loads
            nc.sync.dma_start(out=wc_sb, in_=wc_d[:])
            nc.sync.dma_start(out=wq_sb, in_=wq_d[:])
            nc.sync.dma_start(out=wcq_sb, in_=wcq_d[:])

            def emit_inputs(b, chunked):
                q_sb = ins.tile([P, NJ, D + 1], F16, tag="q_sb")
                qv = q_d[b].rearrange("(t p) d -> p t d", p=P)
                if chunked:
                    nc.gpsimd.dma_start(out=q_sb[:, 0:2, 0:D], in_=qv[:, 0:2])
                    nc.gpsimd.dma_start(out=q_sb[:, 2:4, 0:D], in_=qv[:, 2:4])
                else:
                    nc.gpsimd.dma_start(out=q_sb[:, :, 0:D], in_=qv)
                nc.vector.memset(q_sb[:, :, D : D + 1], 1.0)
                c_sb = ins.tile([P, NT, D], F16, tag="c_sb")
                cv = c_d[b].rearrange("(t p) d -> p t d", p=P)
                nc.gpsimd.dma_start(out=c_sb, in_=cv)
                ov = out_d[b].rearrange("(t p) x -> p t x", p=P)
                return c_sb, q_sb, ov

            make_identity(nc, ident_h)
            nc.vector.memset(ones_f, 1.0)
            nc.vector.memset(ones_h1, 1.0)
            nc.vector.memset(neg_shift, -2.5)
            pending = [emit_inputs(0, True)]

            for b in range(bs):
                c_sb, q_sb, ov = pending.pop(0)
                # prefetch up to two batches ahead
                if b == 0:
                    for nb in (1, 2):
                        if nb < bs:
                            pending.append(emit_inputs(nb, False))
                elif b + 2 < bs:
                    pending.append(emit_inputs(b + 2, False))

                # block0 (= c) stores straight from c_sb; issued here (not at
                # prefetch time) so the sync queue never waits on a future
                # batch's load in front of this batch's stores
                nc.sync.dma_start(out=ov[:, :, 0:D], in_=c_sb)

                # ---------------- transposes (PE, fp16) ----------------
                # qT: both d-chunks into one [P,1024] PSUM, single DVE evac
                qT = work.tile([P, ND, QL], F16, tag="qT")
                psq = ps_tr.tile([P, ND, QL], F16, tag="tr")
                for dc in range(ND):
                    for jc in range(NJ):
                        nc.tensor.transpose(
                            psq[:, dc, ts(jc, P)], q_sb[:, jc, ts(dc, P)], ident_h
                        )
                nc.vector.tensor_copy(qT, psq)

                # c: per d-chunk both halves into one [P,1024] PSUM;
                # chatT (affine) evac on DVE, plain cT evac on ACT
                cT = work.tile([P, ND, CL], F16, tag="cT")
                chatT = work.tile([P, ND, CL], F16, tag="chatT")
                for dc in range(ND):
                    pst = ps_tr.tile([P, CL], F16, tag="tr")
                    for it in range(NT):
                        nc.tensor.transpose(
                            pst[:, ts(it, P)], c_sb[:, it, ts(dc, P)], ident_h
                        )
                    nc.vector.tensor_scalar(
                        out=chatT[:, dc],
                        in0=pst,
                        scalar1=wcq_sb[:, dc : dc + 1],
                        scalar2=wq_sb[:, dc : dc + 1],
                        op0=Mult,
                        op1=Add,
                    )
                    nc.scalar.copy(cT[:, dc], pst)

                # ---------------- scores: tT + exp ----------------
                eT0 = work.tile([P, NJ, IH], F16, tag="eT0")
                eT1 = work.tile([P, NJ, IH], F16, tag="eT1")
                eT = [eT0, eT1]

                def score_chunk(h, jc):
                    pmm = ps_mm.tile([P, IH], F32, tag="tT")
                    for dc in range(ND):
                        nc.tensor.matmul(
                            pmm,
                            qT[:, dc, ts(jc, P)],
                            chatT[:, dc, ds(h * IH, IH)],
                            start=(dc == 0),
                            stop=(dc == ND - 1),
                        )
                    nc.scalar.activation(eT[h][:, jc], pmm, Exp)

                for jc in range(NJ):
                    score_chunk(0, jc)

                for jc in range(NJ):
                    score_chunk(1, jc)
                    if jc == 1:
                        # row max over j, stage 1: max across j-chunks (DVE)
                        M1a0 = work.tile([P, 2, IH], F16, tag="m1a0")
                        nc.vector.tensor_max(
                            M1a0, eT[0][:, 0:2, :], eT[0][:, 2:4, :]
                        )
                        M1h0 = work.tile([P, IH], F16, tag="m1h0")
                        nc.vector.tensor_max(M1h0, M1a0[:, 0, :], M1a0[:, 1, :])

                # sc0 in column layout: 16 tiny N=1 matmuls.  Emitted after
                # the h1 scores so the cT evacuation (ACT) has plenty of
                # slack before the first LDWEIGHTS reads it.
                psc = ps_tr.tile([P, NT], F32, tag="tr")
                for it in range(NT):
                    for dc in range(ND):
                        nc.tensor.matmul(
                            psc[:, it : it + 1],
                            cT[:, dc, ts(it, P)],
                            wc_sb[:, dc : dc + 1],
                            start=(dc == 0),
                            stop=(dc == ND - 1),
                        )
                e_sc0 = work.tile([P, NT], F16, tag="esc0")
                nc.scalar.activation(e_sc0, psc, Exp, bias=neg_shift[:, 0:1])

                # ------------ c2q + row-max + q2c, interleaved ------------
                stage = stg.tile([P, NT, D2], F16, tag="stage")
                c4st = stg.tile([P, NT, D], F16, tag="c4st")
                linv = work.tile([P, NT], F32, tag="linv")
                Me16 = work.tile([P, NT], F16, tag="me")

                def mm2_tile(h, k):
                    it = h * KPH + k
                    po = ps_s.tile([P, D + 1], F32, tag="po")
                    for jc in range(NJ):
                        nc.tensor.matmul(
                            po,
                            eT[h][:, jc, ts(k, P)],
                            q_sb[:, jc],
                            start=(jc == 0),
                            stop=(jc == NJ - 1),
                        )
                    nc.vector.reciprocal(linv[:, it : it + 1], po[:, D : D + 1])
                    nc.scalar.mul(
                        stage[:, it, 0:D], po[:, 0:D], linv[:, it : it + 1]
                    )

                def m1t(h, m1h_tile):
                    # stage 2: transpose the [j,512] partial maxes, then a
                    # free-dim reduce gives m in column layout
                    ptm = ps_tr.tile([P, KPH, P], F16, tag="tr")
                    for k in range(KPH):
                        nc.tensor.transpose(
                            ptm[:, k, :], m1h_tile[:, ts(k, P)], ident_h
                        )
                    nc.vector.reduce_max(
                        Me16[:, h * KPH : (h + 1) * KPH], ptm, AxX
                    )

                mm2_tile(0, 0)
                m1t(0, M1h0)
                mm2_tile(0, 1)
                # chunk-max h1 (eT1 complete by now), then its transposes
                M1a1 = work.tile([P, 2, IH], F16, tag="m1a1")
                nc.vector.tensor_max(M1a1, eT[1][:, 0:2, :], eT[1][:, 2:4, :])
                M1h1 = work.tile([P, IH], F16, tag="m1h1")
                nc.vector.tensor_max(M1h1, M1a1[:, 0, :], M1a1[:, 1, :])
                m1t(1, M1h1)
                # bvec numerators: ebv = (max_j e^t) * e^(sc0-2.5)
                ebv = work.tile([P, NT], F16, tag="ebv")
                nc.vector.tensor_mul(ebv, Me16, e_sc0)
                colsum = work.tile([P, 1], F32, tag="colsum")
                nc.vector.reduce_sum(colsum, ebv, AxX)
                mm2_tile(0, 2)
                mm2_tile(0, 3)
                ps_tot = ps_q.tile([P, 1], F32, tag="q")
                nc.tensor.matmul(ps_tot, ones_f, colsum, start=True, stop=True)
                totinv = work.tile([P, 1], F32, tag="totinv")
                nc.vector.reciprocal(totinv, ps_tot)
                ps_q2c = ps_q.tile([1, D], F32, tag="q")
                for it in range(NT):
                    nc.tensor.matmul(
                        ps_q2c,
                        ebv[:, it : it + 1],
                        c_sb[:, it],
                        start=(it == 0),
                        stop=(it == NT - 1),
                    )
                q2c_row = work.tile([1, D], F16, tag="q2cr")
                nc.vector.tensor_scalar_mul(q2c_row, ps_q2c, totinv[0:1, 0:1])
                # c*c2q h0 (one strided DVE op over 4 tiles), store h0
                nc.vector.tensor_mul(
                    stage[:, 0:KPH, D:D2],
                    c_sb[:, 0:KPH],
                    stage[:, 0:KPH, 0:D],
                )
                nc.sync.dma_start(out=ov[:, 0:KPH, D:D3], in_=stage[:, 0:KPH])
                mm2_tile(1, 0)
                ps_q2cb = ps_q.tile([P, D], F32, tag="q")
                nc.tensor.matmul(
                    ps_q2cb, ones_h1, q2c_row, start=True, stop=True
                )
                q2c_sb = work.tile([P, D], F16, tag="q2csb")
                nc.scalar.copy(q2c_sb, ps_q2cb)
                mm2_tile(1, 1)

                # c*q2c: gpsimd, except split with DVE on the last batch
                # (parallel finish matters only at the tail)
                def c4_mul(it):
                    eng = (
                        nc.vector
                        if (b == bs - 1 and it % 2 == 1)
                        else nc.gpsimd
                    )
                    eng.tensor_mul(c4st[:, it], c_sb[:, it], q2c_sb)

                for it in range(KPH):
                    c4_mul(it)
                nc.sync.dma_start(
                    out=ov[:, 0:KPH, D3 : 4 * D], in_=c4st[:, 0:KPH]
                )
                mm2_tile(1, 2)
                for it in range(KPH, NT):
                    c4_mul(it)
                nc.sync.dma_start(
                    out=ov[:, KPH:NT, D3 : 4 * D], in_=c4st[:, KPH:NT]
                )
                mm2_tile(1, 3)
                nc.vector.tensor_mul(
                    stage[:, KPH:NT, D:D2],
                    c_sb[:, KPH:NT],
                    stage[:, KPH:NT, 0:D],
                )
                # last store on the scalar HWDGE ring so its transfer runs in
                # parallel with the c4 store on the sync ring at the tail
                nc.scalar.dma_start(out=ov[:, KPH:NT, D:D3], in_=stage[:, KPH:NT])

    nc.compile()
    return nc


_NC_CACHE = {}


def _get_nc(bs: int = BS):
    if bs not in _NC_CACHE:
        _NC_CACHE[bs] = build_bass(bs)
    return _NC_CACHE[bs]


def _param_maps(w_c, w_q, w_cq):
    wc_cols = np.ascontiguousarray(
        np.asarray(w_c, np.float32).reshape(ND, P).T.astype(np.float16)
    )
    wq_cols = np.ascontiguousarray(np.asarray(w_q, np.float32).reshape(ND, P).T)
    wcq_cols = np.ascontiguousarray(
        np.asarray(w_cq, np.float32).reshape(ND, P).T
    )
    return wc_cols, wq_cols, wcq_cols


def _run(c, q, w_c, w_q, w_cq, trace=False, **trace_kwargs):
    from concourse.bass_utils import run_bass_kernel_spmd

    c = np.asarray(c, np.float32)
    q = np.asarray(q, np.float32)
    wc_cols, wq_cols, wcq_cols = _param_maps(w_c, w_q, w_cq)

    nc = _get_nc(BS)
    in_maps = []
    for k in range(NCORES):
        in_maps.append(
            {
                "c": np.ascontiguousarray(c[k * BS : (k + 1) * BS]),
                "q": np.ascontiguousarray(q[k * BS : (k + 1) * BS]),
                "wc_cols": wc_cols,
                "wq_cols": wq_cols,
                "wcq_cols": wcq_cols,
            }
        )
    res = None
    last_err = None
    for attempt in range(3):
        try:
            res = run_bass_kernel_spmd(
                nc,
                in_maps,
                core_ids=list(range(NCORES)),
                trace=trace,
                **trace_kwargs,
            )
            break
        except Exception as e:  # transient device wedges clear on retry
            last_err = e
            if "UNRECOVERABLE" not in str(e) and "UNAVAILABLE" not in str(e):
                raise
    if res is None:
        raise last_err
    out = np.concatenate([res.results[k]["out"] for k in range(NCORES)], axis=0)
    return np.ascontiguousarray(out.astype(np.float32)), res


def kernel(c, q, w_c, b_c, w_q, b_q, w_cq, b_cq):
    # b_c/b_q/b_cq provably cancel in both softmaxes; output doesn't use them.
    out, _ = _run(c, q, w_c, w_q, w_cq)
    return out
